# revision 15
# baseline (speedup 1.0000x reference)
"""Trainium2 Bass kernel for nn_Mismatch_loss (weighted per-channel MSE loss).

Contract: kernel(**inputs) takes FULL fp32 inputs (net_out, target,
max_positiones of shape [8, 16, 384, 384]) and returns the FULL scalar
output, distributing work across 8 NeuronCores internally (data-parallel
over batch: core b processes image b).

Math per (b, c) channel (spatial reductions over 384*384 = HW elements):
    d   = t - n
    S1  = sum(t)
    S2  = sum(d^2)
    S3  = sum(d^2 * t)
    loss = ALPHA*S3/(S1+eps) + (1-ALPHA)*(S2-S3)/(HW-S1+eps)
Final [B, C] -> scalar runs on host from the gathered per-channel sums.

Design (from perfetto analysis):
  - Input-stream-bound: both tensors ship as bf16 over the sync HWDGE
    ring at the ~358 GB/s HBM-per-core limit (26.4 us of streaming),
    t/n triggers interleaved per channel-group so each channel's pair
    lands together and compute starts early.
  - All SBUF compute is bf16 (DVE tensor_tensor runs 2x; PE runs
    full-rate bf16).
  - Engine balance: DVE does subs + muls (+ two squares), ACT does the
    other squares with fused per-partition accumulation (S2 columns),
    PE does one-hot column-sum matmuls: S1 (psum1), S3 (psum3), and S2
    for the DVE-squared channels (psum2, reduced mid-stream).
  - Tail: PE t-passes lead p-passes so psum1 closes right after the
    stream ends and its reduce overlaps the last channel's chase; the
    final output DMA is issued from the scalar queue directly after the
    last PSUM reduce (no extra SP semaphore hop).
"""

import os
import sys

import numpy as np
import ml_dtypes

for _p in ("/opt/trn_rl_repo", "/root/.axon_site/_ro/trn_rl_repo"):
    if os.path.isdir(_p) and _p not in sys.path:
        sys.path.append(_p)

B, C, H, W = 8, 16, 384, 384
HWE = H * W          # 147456 spatial elements per channel
P = 128              # SBUF partitions
F = HWE // P         # 1152 elements per partition per channel
SMOOTH = 1e-6
ALPHA = 0.05

RING = 4

# slots whose square runs on DVE (d*d tensor_tensor); their S2 goes
# through PE one-hot passes into psum2 rows 0..len-1
DVE_SQ = (5, 8)

# output column layout in out_all [P, OUT_W] fp32
S2PSUM_COL = 16      # rows 0:len(DVE_SQ) = psum2 reduce (S2 of DVE_SQ slots)
S1_COL = 17          # rows 0:16 = psum1 reduce (sum t per channel)
S3PSUM_COL = 18      # rows 0:16 = psum3 reduce (sum d2*t per channel)
OUT_W = 19

GROUPS = [[0], [1], [2, 3], [4, 5], [6, 7], [8, 9], [10, 11], [12, 13], [14], [15]]

_CACHE = {}


def _build_v2(dma_mode):
    import concourse.bass as bass
    import concourse.mybir as mybir

    bf = mybir.dt.bfloat16
    f32 = mybir.dt.float32
    Alu = mybir.AluOpType
    Act = mybir.ActivationFunctionType

    nc = bass.Bass("TRN2", target_bir_lowering=False, debug=False, num_devices=1)
    t_in = nc.dram_tensor("t_in", [C, P, F], bf, kind="ExternalInput")
    n_in = nc.dram_tensor("n_in", [C, P, F], bf, kind="ExternalInput")
    oneh_in = nc.dram_tensor("oneh", [P, 16, 16], bf, kind="ExternalInput")
    out_all = nc.dram_tensor("out_all", [P, OUT_W], f32, kind="ExternalOutput")

    grp_of = {}
    for g, chans in enumerate(GROUPS):
        for c in chans:
            grp_of[c] = g
    NG = len(GROUPS)

    from contextlib import ExitStack

    with ExitStack() as ctx:
        ctx.enter_context(nc.cleanup_on_exit())
        sb = lambda name, shape, dtype: ctx.enter_context(  # noqa: E731
            nc.sbuf_tensor(name, shape, dtype)
        )
        t_sb = {g: sb(f"t_sb{g}", [P, len(ch), F], bf) for g, ch in enumerate(GROUPS)}
        n_sb = {g: sb(f"n_sb{g}", [P, len(ch), F], bf) for g, ch in enumerate(GROUPS)}
        d_sb = [sb(f"d_sb{k}", [P, F], bf) for k in range(RING)]
        d2_sb = [sb(f"d2_sb{k}", [P, F], bf) for k in range(RING)]
        p_sb = [sb(f"p_sb{k}", [P, F], bf) for k in range(RING)]
        oneh = sb("oneh_sb", [P, 16, 16], bf)
        outb = sb("outb_sb", [P, OUT_W], f32)
        scratch = sb("scratch_sb", [P, 1], bf)
        red_scr = sb("red_scr_sb", [16, 512], f32)
        psum1 = ctx.enter_context(nc.psum_tensor("psum1", [16, 512], f32))
        psum3 = ctx.enter_context(nc.psum_tensor("psum3", [16, 512], f32))
        psum2 = ctx.enter_context(nc.psum_tensor("psum2", [16, 512], f32))

        sem = nc.alloc_semaphore
        s_t = [sem(f"s_t{g}") for g in range(NG)]
        s_n = [sem(f"s_n{g}") for g in range(NG)]
        s_oneh = sem("s_oneh")
        s_d = sem("s_d")       # DVE subs
        s_sqa = sem("s_sqa")   # ACT squares done (ACT queue order)
        s_sqd = sem("s_sqd")   # DVE squares done (DVE_SQ order)
        s_p = sem("s_p")       # DVE muls done (slot order)
        s_pet = sem("s_pet")   # PE t-pass slots completed
        s_pep = sem("s_pep")   # PE p-pass slots completed
        s_ped2 = sem("s_ped2")  # PE d2-pass (DVE_SQ) completed
        s_red = sem("s_red")   # psum reduces completed
        s_out = sem("s_out")

        all_slots = list(range(C))
        act_sq_order = [s for s in all_slots if s not in DVE_SQ]

        sqa_pos = {s: i for i, s in enumerate(act_sq_order)}
        sqd_pos = {s: i for i, s in enumerate(DVE_SQ)}

        def sq_done_wait(engine, slot):
            if slot in DVE_SQ:
                engine.wait_ge(s_sqd, sqd_pos[slot] + 1)
            else:
                engine.wait_ge(s_sqa, sqa_pos[slot] + 1)

        def d2_consumed_wait(engine, slot):
            """d2 ring WAR: wait until slot's d2 consumers are done."""
            engine.wait_ge(s_p, slot + 1)
            if slot in DVE_SQ:
                engine.wait_ge(s_ped2, sqd_pos[slot] + 1)

        def t_ap(s):
            g = grp_of[s]
            return t_sb[g][:, GROUPS[g].index(s), :]

        def n_ap(s):
            g = grp_of[s]
            return n_sb[g][:, GROUPS[g].index(s), :]

        def d_ap(s):
            return d_sb[s % RING][:, :]

        def d2_ap(s):
            return d2_sb[s % RING][:, :]

        def p_ap(s):
            return p_sb[s % RING][:, :]

        # ---- input DMAs (sync HWDGE ring; t/n interleaved per group) ----
        def t_dma(g):
            c0 = GROUPS[g][0]
            nc.sync.dma_start(
                t_sb[g][:, :, :],
                t_in.ap()[c0 : c0 + len(GROUPS[g])].rearrange("c p f -> p c f"),
            ).then_inc(s_t[g], 16)

        def n_dma(g):
            c0 = GROUPS[g][0]
            nc.sync.dma_start(
                n_sb[g][:, :, :],
                n_in.ap()[c0 : c0 + len(GROUPS[g])].rearrange("c p f -> p c f"),
            ).then_inc(s_n[g], 16)

        t_dma(0)
        n_dma(0)
        nc.sync.dma_start(oneh[:, :, :], oneh_in.ap()).then_inc(s_oneh, 16)
        for g in range(1, NG):
            t_dma(g)
            n_dma(g)

        # ---- DVE: subs + muls (+ DVE_SQ squares), interleaved ----
        def emit_sub(s):
            g = grp_of[s]
            if s == GROUPS[g][0]:
                nc.vector.wait_ge(s_t[g], 16)
                nc.vector.wait_ge(s_n[g], 16)
            if s >= RING:
                sq_done_wait(nc.vector, s - RING)  # d ring WAR
            nc.vector.tensor_tensor(
                d_ap(s), t_ap(s), n_ap(s), Alu.subtract
            ).then_inc(s_d, 1)

        def emit_dve_sq(s):
            if s >= RING:
                d2_consumed_wait(nc.vector, s - RING)
            nc.vector.tensor_tensor(
                d2_ap(s), d_ap(s), d_ap(s), Alu.mult
            ).then_inc(s_sqd, 1)

        def emit_mul(s):
            if s not in DVE_SQ:
                sq_done_wait(nc.vector, s)
            if s >= RING:
                nc.vector.wait_ge(s_pep, s - RING + 1)  # p ring WAR
            nc.vector.tensor_tensor(
                p_ap(s), d2_ap(s), t_ap(s), Alu.mult
            ).then_inc(s_p, 1)

        SKEW = 2
        mi = 0
        for i, s in enumerate(all_slots):
            emit_sub(s)
            if s in DVE_SQ:
                emit_dve_sq(s)
            while mi < len(all_slots) and all_slots[mi] <= i - SKEW:
                emit_mul(all_slots[mi])
                mi += 1
        while mi < len(all_slots):
            emit_mul(all_slots[mi])
            mi += 1

        # ---- ACT: table preload, squares w/ accumulation, psum reduces ----
        nc.scalar.activation(scratch[:, :], scratch[:, :], Act.Square)
        psum2_red_after = 12  # emit psum2 reduce after this slot's square
        for s in act_sq_order:
            nc.scalar.wait_ge(s_d, s + 1)
            if s >= RING:
                d2_consumed_wait(nc.scalar, s - RING)  # d2 ring WAR
            nc.scalar.activation(
                d2_ap(s),
                d_ap(s),
                Act.Square,
                accum_out=outb[:, s : s + 1],
            ).then_inc(s_sqa, 1)
            if s == psum2_red_after and DVE_SQ:
                nc.scalar.wait_ge(s_ped2, len(DVE_SQ))
                nc.scalar.activation(
                    red_scr[0 : len(DVE_SQ), :],
                    psum2[0 : len(DVE_SQ), :],
                    Act.Copy,
                    accum_out=outb[0 : len(DVE_SQ), S2PSUM_COL : S2PSUM_COL + 1],
                ).then_inc(s_red, 1)
        nc.scalar.wait_ge(s_pet, C)
        nc.scalar.activation(
            red_scr[:, :], psum1[:, :], Act.Copy,
            accum_out=outb[0:16, S1_COL : S1_COL + 1],
        ).then_inc(s_red, 1)
        nc.scalar.wait_ge(s_pep, C)
        nc.scalar.activation(
            red_scr[:, :], psum3[:, :], Act.Copy,
            accum_out=outb[0:16, S3PSUM_COL : S3PSUM_COL + 1],
        ).then_inc(s_red, 1)
        # final output DMA straight from the scalar queue (HWDGE)
        nc.scalar.dma_start(out_all.ap(), outb[:, :]).then_inc(s_out, 16)

        # ---- PE: one-hot column-sum matmuls; t-passes lead ----
        CHUNKS = (512, 512, 128)

        def emit_pe_pass(s, psum, src_ap, first, last, sem_, w_idx=None):
            w = oneh[:, s if w_idx is None else w_idx, :]
            off = 0
            for wdt in CHUNKS:
                mm = nc.tensor.matmul(
                    psum[:, 0:wdt],
                    lhsT=w,
                    rhs=src_ap[:, off : off + wdt],
                    start=(first and off == 0),
                    stop=(last and off + wdt == F),
                    skip_group_check=True,
                )
                off += wdt
            mm.then_inc(sem_, 1)

        def emit_pe_t(s):
            nc.tensor.wait_ge(s_t[grp_of[s]], 16)
            emit_pe_pass(s, psum1, t_ap(s), s == 0, s == C - 1, s_pet)

        def emit_pe_p(s):
            nc.tensor.wait_ge(s_p, s + 1)
            emit_pe_pass(s, psum3, p_ap(s), s == 0, s == C - 1, s_pep)

        def emit_pe_d2(s):
            nc.tensor.wait_ge(s_sqd, sqd_pos[s] + 1)
            emit_pe_pass(
                s, psum2, d2_ap(s),
                first=(sqd_pos[s] == 0), last=(sqd_pos[s] == len(DVE_SQ) - 1),
                sem_=s_ped2, w_idx=sqd_pos[s],
            )

        nc.tensor.wait_ge(s_oneh, 16)
        pi = 0
        for i, s in enumerate(all_slots):
            emit_pe_t(s)
            if s in DVE_SQ:
                emit_pe_d2(s)
            while pi < len(all_slots) and all_slots[pi] <= i - 2:
                emit_pe_p(all_slots[pi])
                pi += 1
        while pi < len(all_slots):
            emit_pe_p(all_slots[pi])
            pi += 1

        # ---- SP: wait for the output to land ----
        nc.sync.wait_ge(s_out, 16)

    return nc


def _get_nc():
    mode = os.environ.get("BASS_V2_DMA", "bf16")
    key = f"v2_{mode}"
    if key not in _CACHE:
        _CACHE[key] = _build_v2(mode)
    return _CACHE[key]


def _make_oneh():
    oneh = np.zeros((P, 16, 16), dtype=ml_dtypes.bfloat16)
    for c in range(C):
        oneh[:, c, c] = 1.0
    return oneh


def kernel(net_out, target, max_positiones):
    from concourse import bass_utils

    nc = _get_nc()

    t32 = np.ascontiguousarray(np.asarray(target, np.float32).reshape(B, C, P, F))
    n32 = np.ascontiguousarray(np.asarray(net_out, np.float32).reshape(B, C, P, F))
    t_h = t32.astype(ml_dtypes.bfloat16)
    n_h = n32.astype(ml_dtypes.bfloat16)
    oneh = _make_oneh()

    in_maps = [
        {"t_in": t_h[b], "n_in": n_h[b], "oneh": oneh} for b in range(B)
    ]

    last_err = None
    for _attempt in range(4):
        try:
            res = bass_utils.run_bass_kernel_spmd(
                nc, in_maps, core_ids=list(range(8))
            )
            break
        except Exception as e:  # noqa: BLE001
            last_err = e
            import time as _time

            _time.sleep(3.0)
            try:
                import jax

                jax.clear_caches()
                jax.extend.backend.clear_backends()
            except Exception:  # noqa: BLE001
                pass
            _time.sleep(2.0)
    else:
        raise last_err

    S1 = np.empty((B, C), np.float64)
    S2 = np.empty((B, C), np.float64)
    S3 = np.empty((B, C), np.float64)
    for b in range(B):
        out = np.asarray(res.results[b]["out_all"], dtype=np.float64)
        S1[b] = out[:16, S1_COL]
        S3[b] = out[:16, S3PSUM_COL]
        for s in range(C):
            if s in DVE_SQ:
                S2[b, s] = out[DVE_SQ.index(s), S2PSUM_COL]
            else:
                S2[b, s] = out[:, s].sum()

    m1, m2, d1 = S3, S2 - S3, S1
    d2n = float(HWE) - d1
    loss = ALPHA * m1 / (d1 + SMOOTH) + (1.0 - ALPHA) * m2 / (d2n + SMOOTH)

    # active-mask: S1 != 0 implies max(target[b,c]) != 0 for non-negative
    # targets; the S1 == 0 corner is resolved exactly on host.
    active = S1 != 0.0
    for b, c in zip(*np.nonzero(~active)):
        mt = np.max(target[b, c])
        mmp = np.max(max_positiones[b, c])
        active[b, c] = not (mt == 0.0 and mmp == 0.0)

    losses = np.where(active, loss, 0.0)
    count = (losses != 0.0).sum(axis=1).astype(np.float64)
    img_losses = losses.sum(axis=1) / count
    return np.float32(img_losses.mean())


# revision 16
# speedup vs baseline: 1.1224x; 1.1224x over previous
"""Trainium2 Bass kernel for nn_Mismatch_loss (weighted per-channel MSE loss).

Contract: kernel(**inputs) takes FULL fp32 inputs (net_out, target,
max_positiones of shape [8, 16, 384, 384]) and returns the FULL scalar
output, distributing work across 8 NeuronCores internally (data-parallel
over batch: core b processes image b).

Math per (b, c) channel (spatial reductions over 384*384 = HW elements):
    d   = t - n
    S1  = sum(t)
    S2  = sum(d^2)
    S3  = sum(d^2 * t)
    loss = ALPHA*S3/(S1+eps) + (1-ALPHA)*(S2-S3)/(HW-S1+eps)
Final [B, C] -> scalar runs on host from the gathered per-channel sums.

Design (from perfetto analysis):
  - Input-stream-bound: both tensors ship as bf16 over the sync HWDGE
    ring at the ~358 GB/s HBM-per-core limit (26.4 us of streaming),
    t/n triggers interleaved per channel-group so each channel's pair
    lands together and compute starts early.
  - All SBUF compute is bf16 (DVE tensor_tensor runs 2x; PE runs
    full-rate bf16).
  - Engine balance: DVE does subs + muls (+ two squares), ACT does the
    other squares with fused per-partition accumulation (S2 columns),
    PE does one-hot column-sum matmuls: S1 (psum1), S3 (psum3), and S2
    for the DVE-squared channels (psum2, reduced mid-stream).
  - Tail: PE t-passes lead p-passes so psum1 closes right after the
    stream ends and its reduce overlaps the last channel's chase; the
    final output DMA is issued from the scalar queue directly after the
    last PSUM reduce (no extra SP semaphore hop).
"""

import os
import sys

import numpy as np
import ml_dtypes

for _p in ("/opt/trn_rl_repo", "/root/.axon_site/_ro/trn_rl_repo"):
    if os.path.isdir(_p) and _p not in sys.path:
        sys.path.append(_p)

B, C, H, W = 8, 16, 384, 384
HWE = H * W          # 147456 spatial elements per channel
P = 128              # SBUF partitions
F = HWE // P         # 1152 elements per partition per channel
SMOOTH = 1e-6
ALPHA = 0.05

RING = 6

# slots whose square runs on DVE (d*d tensor_tensor); their S2 goes
# through PE one-hot passes into psum2 rows 0..len-1
DVE_SQ = (5, 8)

# output column layout in out_all [P, OUT_W] fp32
S2PSUM_COL = 16      # rows 0:len(DVE_SQ) = psum2 reduce (S2 of DVE_SQ slots)
S1_COL = 17          # rows 0:16 = psum1 reduce (sum t per channel)
S3PSUM_COL = 18      # rows 0:16 = psum3 reduce (sum d2*t per channel)
OUT_W = 19

GROUPS = [[0], [1], [2], [3], [4, 5], [6, 7], [8, 9], [10, 11], [12, 13], [14, 15]]

_CACHE = {}


def _build_v2(dma_mode):
    import concourse.bass as bass
    import concourse.mybir as mybir

    bf = mybir.dt.bfloat16
    f32 = mybir.dt.float32
    Alu = mybir.AluOpType
    Act = mybir.ActivationFunctionType

    nc = bass.Bass("TRN2", target_bir_lowering=False, debug=False, num_devices=1)
    t_in = nc.dram_tensor("t_in", [C, P, F], bf, kind="ExternalInput")
    n_in = nc.dram_tensor("n_in", [C, P, F], bf, kind="ExternalInput")
    oneh_in = nc.dram_tensor("oneh", [P, 16, 16], bf, kind="ExternalInput")
    out_all = nc.dram_tensor("out_all", [P, OUT_W], f32, kind="ExternalOutput")

    grp_of = {}
    for g, chans in enumerate(GROUPS):
        for c in chans:
            grp_of[c] = g
    NG = len(GROUPS)

    from contextlib import ExitStack

    with ExitStack() as ctx:
        ctx.enter_context(nc.cleanup_on_exit())
        sb = lambda name, shape, dtype: ctx.enter_context(  # noqa: E731
            nc.sbuf_tensor(name, shape, dtype)
        )
        t_sb = {g: sb(f"t_sb{g}", [P, len(ch), F], bf) for g, ch in enumerate(GROUPS)}
        n_sb = {g: sb(f"n_sb{g}", [P, len(ch), F], bf) for g, ch in enumerate(GROUPS)}
        d_sb = [sb(f"d_sb{k}", [P, F], bf) for k in range(RING)]
        d2_sb = [sb(f"d2_sb{k}", [P, F], bf) for k in range(RING)]
        p_sb = [sb(f"p_sb{k}", [P, F], bf) for k in range(RING)]
        oneh = sb("oneh_sb", [P, 16, 16], bf)
        outb = sb("outb_sb", [P, OUT_W], f32)
        scratch = sb("scratch_sb", [P, 1], bf)
        red_scr = sb("red_scr_sb", [16, 512], f32)
        psum1 = ctx.enter_context(nc.psum_tensor("psum1", [16, 512], f32))
        psum3 = ctx.enter_context(nc.psum_tensor("psum3", [16, 512], f32))
        psum2 = ctx.enter_context(nc.psum_tensor("psum2", [16, 512], f32))

        sem = nc.alloc_semaphore
        s_t = [sem(f"s_t{g}") for g in range(NG)]
        s_n = [sem(f"s_n{g}") for g in range(NG)]
        s_oneh = sem("s_oneh")
        s_d = sem("s_d")       # DVE subs
        s_sqa = sem("s_sqa")   # ACT squares done (ACT queue order)
        s_sqd = sem("s_sqd")   # DVE squares done (DVE_SQ order)
        s_p = sem("s_p")       # DVE muls done (slot order)
        s_pet = sem("s_pet")   # PE t-pass slots completed
        s_pep = sem("s_pep")   # PE p-pass slots completed
        s_ped2 = sem("s_ped2")  # PE d2-pass (DVE_SQ) completed
        s_red = sem("s_red")   # psum reduces completed
        s_out = sem("s_out")

        all_slots = list(range(C))
        act_sq_order = [s for s in all_slots if s not in DVE_SQ]

        sqa_pos = {s: i for i, s in enumerate(act_sq_order)}
        sqd_pos = {s: i for i, s in enumerate(DVE_SQ)}

        def sq_done_wait(engine, slot):
            if slot in DVE_SQ:
                engine.wait_ge(s_sqd, sqd_pos[slot] + 1)
            else:
                engine.wait_ge(s_sqa, sqa_pos[slot] + 1)

        def d2_consumed_wait(engine, slot):
            """d2 ring WAR: wait until slot's d2 consumers are done."""
            engine.wait_ge(s_p, slot + 1)
            if slot in DVE_SQ:
                engine.wait_ge(s_ped2, sqd_pos[slot] + 1)

        def t_ap(s):
            g = grp_of[s]
            return t_sb[g][:, GROUPS[g].index(s), :]

        def n_ap(s):
            g = grp_of[s]
            return n_sb[g][:, GROUPS[g].index(s), :]

        def d_ap(s):
            return d_sb[s % RING][:, :]

        def d2_ap(s):
            return d2_sb[s % RING][:, :]

        def p_ap(s):
            return p_sb[s % RING][:, :]

        # ---- input DMAs (sync HWDGE ring; t/n interleaved per group) ----
        def t_dma(g):
            c0 = GROUPS[g][0]
            nc.sync.dma_start(
                t_sb[g][:, :, :],
                t_in.ap()[c0 : c0 + len(GROUPS[g])].rearrange("c p f -> p c f"),
            ).then_inc(s_t[g], 16)

        def n_dma(g):
            c0 = GROUPS[g][0]
            nc.sync.dma_start(
                n_sb[g][:, :, :],
                n_in.ap()[c0 : c0 + len(GROUPS[g])].rearrange("c p f -> p c f"),
            ).then_inc(s_n[g], 16)

        t_dma(0)
        n_dma(0)
        nc.sync.dma_start(oneh[:, :, :], oneh_in.ap()).then_inc(s_oneh, 16)
        for g in range(1, NG):
            t_dma(g)
            n_dma(g)

        # ---- DVE: subs + muls (+ DVE_SQ squares), interleaved ----
        def emit_sub(s):
            g = grp_of[s]
            if s == GROUPS[g][0]:
                nc.vector.wait_ge(s_t[g], 16)
                nc.vector.wait_ge(s_n[g], 16)
            if s >= RING:
                sq_done_wait(nc.vector, s - RING)  # d ring WAR
            nc.vector.tensor_tensor(
                d_ap(s), t_ap(s), n_ap(s), Alu.subtract
            ).then_inc(s_d, 1)

        def emit_dve_sq(s):
            if s >= RING:
                d2_consumed_wait(nc.vector, s - RING)
            nc.vector.tensor_tensor(
                d2_ap(s), d_ap(s), d_ap(s), Alu.mult
            ).then_inc(s_sqd, 1)

        def emit_mul(s):
            if s not in DVE_SQ:
                sq_done_wait(nc.vector, s)
            if s >= RING:
                nc.vector.wait_ge(s_pep, s - RING + 1)  # p ring WAR
            nc.vector.tensor_tensor(
                p_ap(s), d2_ap(s), t_ap(s), Alu.mult
            ).then_inc(s_p, 1)

        SKEW = 2
        mi = 0
        for i, s in enumerate(all_slots):
            emit_sub(s)
            if s in DVE_SQ:
                emit_dve_sq(s)
            while mi < len(all_slots) and all_slots[mi] <= i - SKEW:
                emit_mul(all_slots[mi])
                mi += 1
        while mi < len(all_slots):
            emit_mul(all_slots[mi])
            mi += 1

        # ---- ACT: table preload, squares w/ accumulation, psum reduces ----
        nc.scalar.activation(scratch[:, :], scratch[:, :], Act.Square)
        psum2_red_after = 12  # emit psum2 reduce after this slot's square
        for s in act_sq_order:
            nc.scalar.wait_ge(s_d, s + 1)
            if s >= RING:
                d2_consumed_wait(nc.scalar, s - RING)  # d2 ring WAR
            nc.scalar.activation(
                d2_ap(s),
                d_ap(s),
                Act.Square,
                accum_out=outb[:, s : s + 1],
            ).then_inc(s_sqa, 1)
            if s == psum2_red_after and DVE_SQ:
                nc.scalar.wait_ge(s_ped2, len(DVE_SQ))
                nc.scalar.activation(
                    red_scr[0 : len(DVE_SQ), :],
                    psum2[0 : len(DVE_SQ), :],
                    Act.Copy,
                    accum_out=outb[0 : len(DVE_SQ), S2PSUM_COL : S2PSUM_COL + 1],
                ).then_inc(s_red, 1)
        nc.scalar.wait_ge(s_pet, C)
        nc.scalar.activation(
            red_scr[:, :], psum1[:, :], Act.Copy,
            accum_out=outb[0:16, S1_COL : S1_COL + 1],
        ).then_inc(s_red, 1)
        nc.scalar.wait_ge(s_pep, C)
        nc.scalar.activation(
            red_scr[:, :], psum3[:, :], Act.Copy,
            accum_out=outb[0:16, S3PSUM_COL : S3PSUM_COL + 1],
        ).then_inc(s_red, 1)
        # final output DMA straight from the scalar queue (HWDGE)
        nc.scalar.dma_start(out_all.ap(), outb[:, :]).then_inc(s_out, 16)

        # ---- PE: one-hot column-sum matmuls; t-passes lead ----
        CHUNKS = (512, 512, 128)

        def emit_pe_pass(s, psum, src_ap, first, last, sem_, w_idx=None):
            w = oneh[:, s if w_idx is None else w_idx, :]
            off = 0
            for wdt in CHUNKS:
                mm = nc.tensor.matmul(
                    psum[:, 0:wdt],
                    lhsT=w,
                    rhs=src_ap[:, off : off + wdt],
                    start=(first and off == 0),
                    stop=(last and off + wdt == F),
                    skip_group_check=True,
                )
                off += wdt
            mm.then_inc(sem_, 1)

        def emit_pe_t(s):
            nc.tensor.wait_ge(s_t[grp_of[s]], 16)
            emit_pe_pass(s, psum1, t_ap(s), s == 0, s == C - 1, s_pet)

        def emit_pe_p(s):
            nc.tensor.wait_ge(s_p, s + 1)
            emit_pe_pass(s, psum3, p_ap(s), s == 0, s == C - 1, s_pep)

        def emit_pe_d2(s):
            nc.tensor.wait_ge(s_sqd, sqd_pos[s] + 1)
            emit_pe_pass(
                s, psum2, d2_ap(s),
                first=(sqd_pos[s] == 0), last=(sqd_pos[s] == len(DVE_SQ) - 1),
                sem_=s_ped2, w_idx=sqd_pos[s],
            )

        nc.tensor.wait_ge(s_oneh, 16)
        pi = 0
        for i, s in enumerate(all_slots):
            emit_pe_t(s)
            if s in DVE_SQ:
                emit_pe_d2(s)
            while pi < len(all_slots) and all_slots[pi] <= i - 2:
                emit_pe_p(all_slots[pi])
                pi += 1
        while pi < len(all_slots):
            emit_pe_p(all_slots[pi])
            pi += 1

        # ---- SP: wait for the output to land ----
        nc.sync.wait_ge(s_out, 16)

    return nc


def _get_nc():
    mode = os.environ.get("BASS_V2_DMA", "bf16")
    key = f"v2_{mode}"
    if key not in _CACHE:
        _CACHE[key] = _build_v2(mode)
    return _CACHE[key]


def _make_oneh():
    oneh = np.zeros((P, 16, 16), dtype=ml_dtypes.bfloat16)
    for c in range(C):
        oneh[:, c, c] = 1.0
    return oneh


def kernel(net_out, target, max_positiones):
    from concourse import bass_utils

    nc = _get_nc()

    t32 = np.ascontiguousarray(np.asarray(target, np.float32).reshape(B, C, P, F))
    n32 = np.ascontiguousarray(np.asarray(net_out, np.float32).reshape(B, C, P, F))
    t_h = t32.astype(ml_dtypes.bfloat16)
    n_h = n32.astype(ml_dtypes.bfloat16)
    oneh = _make_oneh()

    in_maps = [
        {"t_in": t_h[b], "n_in": n_h[b], "oneh": oneh} for b in range(B)
    ]

    last_err = None
    for _attempt in range(4):
        try:
            res = bass_utils.run_bass_kernel_spmd(
                nc, in_maps, core_ids=list(range(8))
            )
            break
        except Exception as e:  # noqa: BLE001
            last_err = e
            import time as _time

            _time.sleep(3.0)
            try:
                import jax

                jax.clear_caches()
                jax.extend.backend.clear_backends()
            except Exception:  # noqa: BLE001
                pass
            _time.sleep(2.0)
    else:
        raise last_err

    S1 = np.empty((B, C), np.float64)
    S2 = np.empty((B, C), np.float64)
    S3 = np.empty((B, C), np.float64)
    for b in range(B):
        out = np.asarray(res.results[b]["out_all"], dtype=np.float64)
        S1[b] = out[:16, S1_COL]
        S3[b] = out[:16, S3PSUM_COL]
        for s in range(C):
            if s in DVE_SQ:
                S2[b, s] = out[DVE_SQ.index(s), S2PSUM_COL]
            else:
                S2[b, s] = out[:, s].sum()

    m1, m2, d1 = S3, S2 - S3, S1
    d2n = float(HWE) - d1
    loss = ALPHA * m1 / (d1 + SMOOTH) + (1.0 - ALPHA) * m2 / (d2n + SMOOTH)

    # active-mask: S1 != 0 implies max(target[b,c]) != 0 for non-negative
    # targets; the S1 == 0 corner is resolved exactly on host.
    active = S1 != 0.0
    for b, c in zip(*np.nonzero(~active)):
        mt = np.max(target[b, c])
        mmp = np.max(max_positiones[b, c])
        active[b, c] = not (mt == 0.0 and mmp == 0.0)

    losses = np.where(active, loss, 0.0)
    count = (losses != 0.0).sum(axis=1).astype(np.float64)
    img_losses = losses.sum(axis=1) / count
    return np.float32(img_losses.mean())


# revision 17
# speedup vs baseline: 1.1820x; 1.0530x over previous
"""Trainium2 Bass kernel for nn_Mismatch_loss (weighted per-channel MSE loss).

Contract: kernel(**inputs) takes FULL fp32 inputs (net_out, target,
max_positiones of shape [8, 16, 384, 384]) and returns the FULL scalar
output, distributing work across 8 NeuronCores internally.

Sharding: data-parallel over batch — core b processes image b.

Math per (b, c) channel (spatial reductions over 384*384 = HW elements):
    d   = t - n
    d2  = d * d
    S1  = sum(t)        (= d1 in the reference)
    S2  = sum(d2)       (= m1 + m2)
    S3  = sum(d2 * t)   (= m1)
    loss = ALPHA*S3/(S1+eps) + (1-ALPHA)*(S2-S3)/(HW-S1+eps)
The tiny [B, C] -> scalar finalization (active-mask, count of nonzero
losses, means) runs on host from the gathered per-channel sums.

Device layout per core: channel c is a [128, 1152] tile (partition-major
split of the 147456 spatial elements). Engines:
  - DVE: d = t - n, p = d2 * t      (fp16 tensor_tensor, 2x mode)
  - ACT: d2 = Square(d) with accum_out -> per-partition sum(d2) columns
  - PE : per-channel column sums of t and p via one-hot fp16 weights,
         accumulated across chunks/channels into PSUM [16, 512]
  - fp32 accumulation everywhere (PSUM / accum_out are fp32)

Inputs are cast to fp16 on host before upload: halves HBM traffic (the
kernel is DMA-bound) at ~1e-5 relative error on the final scalar.

max_positiones is only consulted when a channel of target is exactly
all-zero (cannot happen for this problem's random-uniform inputs); that
case is handled exactly on host without shipping the tensor to devices.
"""

import os
import sys

import numpy as np

for _p in ("/opt/trn_rl_repo", "/root/.axon_site/_ro/trn_rl_repo"):
    if os.path.isdir(_p) and _p not in sys.path:
        sys.path.append(_p)

B, C, H, W = 8, 16, 384, 384
HWE = H * W          # 147456 spatial elements per channel
P = 128              # SBUF partitions
F = HWE // P         # 1152 elements per partition per channel
MACRO = 4            # channels per macro tile (per DMA)
N_MACRO = C // MACRO
CHUNKS = (512, 512, 128)   # PE matmul free-dim chunking of F
SMOOTH = 1e-6
ALPHA = 0.05

_CACHE = {}


def _build_bass():
    import concourse.bacc as bacc
    import concourse.mybir as mybir
    from concourse.tile import TileContext

    f16 = mybir.dt.float16
    f32 = mybir.dt.float32
    Alu = mybir.AluOpType

    # num_devices=1: the 8 cores run fully independent SPMD instances (no
    # collectives), so no cross-core barriers are needed.
    nc = bacc.Bacc(
        "TRN2", target_bir_lowering=False, debug=False, num_devices=1
    )
    t_in = nc.dram_tensor("t_in", [C, P, F], f16, kind="ExternalInput")
    n_in = nc.dram_tensor("n_in", [C, P, F], f16, kind="ExternalInput")
    # oneh[p, c, m] = 1.0 where m == c: stationary weights routing channel
    # c's column sums to PSUM partition c.
    oneh_in = nc.dram_tensor("oneh", [P, C, 16], f16, kind="ExternalInput")
    out_s13 = nc.dram_tensor("out_s13", [16, 2], f32, kind="ExternalOutput")
    out_acc2 = nc.dram_tensor("out_acc2", [P, C], f32, kind="ExternalOutput")

    with TileContext(nc) as tc:
        with (
            tc.tile_pool(name="io", bufs=N_MACRO) as io_pool,
            tc.tile_pool(name="cpool", bufs=6) as ch_pool,
            tc.tile_pool(name="consts", bufs=1) as const_pool,
            tc.tile_pool(name="accs", bufs=1) as acc_pool,
            tc.tile_pool(name="ps", bufs=1, space="PSUM") as psum_pool,
        ):
            oneh = const_pool.tile([P, C, 16], f16)
            nc.sync.dma_start(oneh, oneh_in.ap())
            acc2 = acc_pool.tile([P, C], f32)     # per-partition sum(d2), col c
            s13 = acc_pool.tile([16, 2], f32)
            psum1 = psum_pool.tile([16, 512], f32)  # sum(t) partials
            psum3 = psum_pool.tile([16, 512], f32)  # sum(d2*t) partials

            # Prefetch everything: all input DMAs issue up front, so the
            # SDMA engines stream continuously at HBM rate.
            t_tiles, n_tiles = [], []
            for m in range(N_MACRO):
                c0 = m * MACRO
                t_t = io_pool.tile([P, MACRO, F], f16, tag="t")
                n_t = io_pool.tile([P, MACRO, F], f16, tag="n")
                nc.sync.dma_start(
                    t_t, t_in.ap()[c0 : c0 + MACRO].rearrange("c p f -> p c f")
                )
                nc.sync.dma_start(
                    n_t, n_in.ap()[c0 : c0 + MACRO].rearrange("c p f -> p c f")
                )
                t_tiles.append(t_t)
                n_tiles.append(n_t)

            for m in range(N_MACRO):
                c0 = m * MACRO
                t_t, n_t = t_tiles[m], n_tiles[m]
                for lc in range(MACRO):
                    c = c0 + lc
                    d_c = ch_pool.tile([P, F], f16, tag="d")
                    nc.vector.tensor_tensor(
                        d_c, t_t[:, lc, :], n_t[:, lc, :], Alu.subtract
                    )
                    d2_c = ch_pool.tile([P, F], f16, tag="d2")
                    nc.scalar.activation(
                        d2_c,
                        d_c,
                        mybir.ActivationFunctionType.Square,
                        accum_out=acc2[:, c : c + 1],
                    )
                    p_c = ch_pool.tile([P, F], f16, tag="p")
                    nc.vector.tensor_tensor(p_c, d2_c, t_t[:, lc, :], Alu.mult)
                    w = oneh[:, c, :]
                    off = 0
                    for wdt in CHUNKS:
                        first = c == 0 and off == 0
                        last = c == C - 1 and off + wdt == F
                        nc.tensor.matmul(
                            psum1[:, 0:wdt],
                            lhsT=w,
                            rhs=t_t[:, lc, off : off + wdt],
                            start=first,
                            stop=last,
                            skip_group_check=True,
                        )
                        nc.tensor.matmul(
                            psum3[:, 0:wdt],
                            lhsT=w,
                            rhs=p_c[:, off : off + wdt],
                            start=first,
                            stop=last,
                            skip_group_check=True,
                        )
                        off += wdt

            nc.vector.tensor_reduce(
                s13[:, 0:1], psum1, axis=mybir.AxisListType.X, op=Alu.add
            )
            nc.vector.tensor_reduce(
                s13[:, 1:2], psum3, axis=mybir.AxisListType.X, op=Alu.add
            )
            nc.sync.dma_start(out_s13.ap(), s13)
            nc.sync.dma_start(out_acc2.ap(), acc2)

    nc.compile()
    return nc


def _build_bass_raw():
    """Hand-scheduled raw-bass version: same pipeline as the Tile build but
    with manual semaphores and a minimal end-of-kernel protocol, avoiding
    Tile's ~15us of preamble/postamble barriers.

    Engine programs (per core):
      SP  : all input DMAs up front (2-channel granularity), output DMAs
            gated on completion sems.
      DVE : d_c = t_c - n_c and p_c = d2_c * t_c (fp16 2x), subs running
            3 channels ahead of muls; final PSUM->[16,1] reductions.
      ACT : d2_c = Square(d_c) with fused per-partition accumulation.
      PE  : per-channel column sums of t and p into PSUM via one-hot
            weights, t-matmuls leading p-matmuls by one macro.
    """
    import concourse.bass as bass
    import concourse.mybir as mybir

    f16 = mybir.dt.float16
    f32 = mybir.dt.float32
    Alu = mybir.AluOpType
    Act = mybir.ActivationFunctionType

    # Channel groups per input DMA: first channels in 1-channel DMAs so
    # compute starts as early as possible, last channels in 1-channel DMAs
    # so the end-of-stream dependency tail is short, 2-channel in between.
    n_singles = int(os.environ.get("BASS_HEAD_SINGLES", "4"))
    GROUPS = [[c] for c in range(n_singles)] + [
        [c, c + 1] for c in range(n_singles, C, 2)
    ]
    NG = len(GROUPS)
    grp_of = {}
    for g, chans in enumerate(GROUPS):
        for c in chans:
            grp_of[c] = g
    RING = 4                     # d/d2/p ring depth (channels in flight)

    nc = bass.Bass("TRN2", target_bir_lowering=False, debug=False, num_devices=1)
    t_in = nc.dram_tensor("t_in", [C, P, F], f16, kind="ExternalInput")
    n_in = nc.dram_tensor("n_in", [C, P, F], f16, kind="ExternalInput")
    # Single merged output: cols 0..15 = per-partition sum(d2) (acc2);
    # [0:16, 16] = per-channel sum(t); [0:16, 17] = per-channel sum(d2*t).
    out_all = nc.dram_tensor("out_all", [P, C + 2], f32, kind="ExternalOutput")

    from contextlib import ExitStack

    with ExitStack() as ctx:
        ctx.enter_context(nc.cleanup_on_exit())
        sb = lambda name, shape, dtype: ctx.enter_context(  # noqa: E731
            nc.sbuf_tensor(name, shape, dtype)
        )
        t_sb = {}
        n_sb = {}
        for g, chans in enumerate(GROUPS):
            t_sb[g] = sb(f"t_sb{g}", [P, len(chans), F], f16)
            n_sb[g] = sb(f"n_sb{g}", [P, len(chans), F], f16)
        d_sb = [sb(f"d_sb{k}", [P, F], f16) for k in range(RING)]
        d2_sb = [sb(f"d2_sb{k}", [P, F], f16) for k in range(RING)]
        p_sb = [sb(f"p_sb{k}", [P, F], f16) for k in range(RING)]
        oneh = sb("oneh_sb", [P, C, 16], f16)
        outb = sb("outb_sb", [P, C + 2], f32)
        scratch = sb("scratch_sb", [P, 1], f16)
        red_scr = sb("red_scr_sb", [16, 512], f32)
        psum1 = ctx.enter_context(nc.psum_tensor("psum1", [16, 512], f32))
        psum3 = ctx.enter_context(nc.psum_tensor("psum3", [16, 512], f32))

        sem = nc.alloc_semaphore
        s_t = [sem(f"s_t{g}") for g in range(NG)]
        s_n = [sem(f"s_n{g}") for g in range(NG)]
        s_oneh = sem("s_oneh")
        s_d = sem("s_d")      # subs completed
        s_sq = sem("s_sq")    # squares completed
        s_p = sem("s_p")      # muls completed
        s_pet = sem("s_pet")  # PE t-matmul channel groups completed
        s_pep = sem("s_pep")  # PE p-matmul channel groups completed
        s_red = sem("s_red")  # final reductions completed
        s_out = sem("s_out")  # output DMA completed

        def t_ap(c):
            g = grp_of[c]
            return t_sb[g][:, c - GROUPS[g][0], :]

        def n_ap(c):
            g = grp_of[c]
            return n_sb[g][:, c - GROUPS[g][0], :]

        # ---- GPSIMD: build one-hot weights on device (no DMA needed) ----
        nc.gpsimd.memset(oneh[:, :, :], 0.0)
        for c in range(C):
            ms = nc.gpsimd.memset(oneh[:, c, c : c + 1], 1.0)
        ms.then_inc(s_oneh, 1)

        # ---- SP: DMAs ----
        def in_dma(dst, src, chans, s):
            c0 = chans[0]
            nc.sync.dma_start(
                dst[:, :, :],
                src[c0 : c0 + len(chans)].rearrange("c p f -> p c f"),
            ).then_inc(s, 16)

        for g in range(NG):
            in_dma(t_sb[g], t_in.ap(), GROUPS[g], s_t[g])
            in_dma(n_sb[g], n_in.ap(), GROUPS[g], s_n[g])
        # acc2 columns ship as soon as the squares finish (overlaps the
        # final muls/matmuls); the tiny reduction outputs ship last.
        nc.sync.wait_ge(s_sq, C)
        nc.sync.dma_start(
            out_all.ap()[:, 0:C], outb[:, 0:C]
        ).then_inc(s_out, 16)
        nc.sync.wait_ge(s_red, 2)
        nc.sync.dma_start(
            out_all.ap()[0:16, C : C + 2], outb[0:16, C : C + 2]
        ).then_inc(s_out, 16)
        nc.sync.wait_ge(s_out, 32)

        # ---- DVE: subs (3 channels ahead) and muls ----
        def emit_sub(c):
            g = grp_of[c]
            if c == GROUPS[g][0]:
                nc.vector.wait_ge(s_t[g], 16)
                nc.vector.wait_ge(s_n[g], 16)
            nc.vector.tensor_tensor(
                d_sb[c % RING][:, :], t_ap(c), n_ap(c), Alu.subtract
            ).then_inc(s_d, 1)

        def emit_mul(j):
            nc.vector.wait_ge(s_sq, j + 1)
            if j >= RING:
                nc.vector.wait_ge(s_pep, j - (RING - 1))
            nc.vector.tensor_tensor(
                p_sb[j % RING][:, :], d2_sb[j % RING][:, :], t_ap(j), Alu.mult
            ).then_inc(s_p, 1)

        SKEW = 2
        for i in range(C + SKEW):
            if i < C:
                emit_sub(i)
            if i - SKEW >= 0:
                emit_mul(i - SKEW)

        # ---- ACT: squares with fused per-partition accumulation ----
        # Dummy activation first: pulls the one-time ACT_TABLE_LOAD
        # (~1.3us) off the critical path of the first real square.
        nc.scalar.activation(scratch[:, :], scratch[:, :], Act.Square)
        for c in range(C):
            nc.scalar.wait_ge(s_d, c + 1)
            if c >= RING:
                nc.scalar.wait_ge(s_p, c - (RING - 1))
            nc.scalar.activation(
                d2_sb[c % RING][:, :],
                d_sb[c % RING][:, :],
                Act.Square,
                accum_out=outb[:, c : c + 1],
            ).then_inc(s_sq, 1)
        # Final PSUM -> [16,1] reductions.
        if os.environ.get("BASS_REDUCE_ENGINE", "act") == "act":
            nc.scalar.wait_ge(s_pet, C)
            nc.scalar.activation(
                red_scr[:, :], psum1[:, :], Act.Copy,
                accum_out=outb[0:16, C : C + 1],
            ).then_inc(s_red, 1)
            nc.scalar.wait_ge(s_pep, C)
            nc.scalar.activation(
                red_scr[:, :], psum3[:, :], Act.Copy,
                accum_out=outb[0:16, C + 1 : C + 2],
            ).then_inc(s_red, 1)
        else:
            nc.vector.wait_ge(s_pet, C)
            nc.vector.tensor_reduce(
                outb[0:16, C : C + 1], psum1[:, :],
                axis=mybir.AxisListType.X, op=Alu.add,
            ).then_inc(s_red, 1)
            nc.vector.wait_ge(s_pep, C)
            nc.vector.tensor_reduce(
                outb[0:16, C + 1 : C + 2], psum3[:, :],
                axis=mybir.AxisListType.X, op=Alu.add,
            ).then_inc(s_red, 1)

        # ---- PE: one-hot column-sum matmuls; t leads p by one group ----
        def emit_t_mms(g):
            nc.tensor.wait_ge(s_t[g], 16)
            for lc, c in enumerate(GROUPS[g]):
                w = oneh[:, c, :]
                off = 0
                for wdt in CHUNKS:
                    mm = nc.tensor.matmul(
                        psum1[:, 0:wdt],
                        lhsT=w,
                        rhs=t_sb[g][:, lc, off : off + wdt],
                        start=(c == 0 and off == 0),
                        stop=(c == C - 1 and off + wdt == F),
                        skip_group_check=True,
                    )
                    off += wdt
                mm.then_inc(s_pet, 1)

        def emit_p_mms(c):
            nc.tensor.wait_ge(s_p, c + 1)
            w = oneh[:, c, :]
            off = 0
            for wdt in CHUNKS:
                mm = nc.tensor.matmul(
                    psum3[:, 0:wdt],
                    lhsT=w,
                    rhs=p_sb[c % RING][:, off : off + wdt],
                    start=(c == 0 and off == 0),
                    stop=(c == C - 1 and off + wdt == F),
                    skip_group_check=True,
                )
                off += wdt
            mm.then_inc(s_pep, 1)

        nc.tensor.wait_ge(s_oneh, 1)
        emit_t_mms(0)
        done_p = 0
        for g in range(1, NG):
            emit_t_mms(g)
            for c in GROUPS[g - 1]:
                emit_p_mms(c)
                done_p = c
        for c in range(done_p + 1, C):
            emit_p_mms(c)

        nc.all_engine_barrier()

    return nc


def _get_nc():
    impl = os.environ.get("BASS_LOSS_IMPL", "raw")
    key = "nc_{}_{}_{}".format(
        impl,
        os.environ.get("BASS_HEAD_SINGLES", "4"),
        os.environ.get("BASS_REDUCE_ENGINE", "act"),
    )
    if key not in _CACHE:
        _CACHE[key] = _build_bass_raw() if impl == "raw" else _build_bass()
    return _CACHE[key]


def kernel(net_out, target, max_positiones):
    from concourse import bass_utils

    nc = _get_nc()

    t16 = np.asarray(target, dtype=np.float16).reshape(B, C, P, F)
    n16 = np.asarray(net_out, dtype=np.float16).reshape(B, C, P, F)

    expected_inputs = set()
    import concourse.mybir as mybir

    for alloc in nc.m.functions[0].allocations:
        if (
            isinstance(alloc, mybir.MemoryLocationSet)
            and alloc.kind == "ExternalInput"
        ):
            expected_inputs.add(alloc.memorylocations[0].name)

    def make_map(b):
        m = {"t_in": t16[b], "n_in": n16[b]}
        if "oneh" in expected_inputs:
            oneh = np.zeros((P, C, 16), dtype=np.float16)
            for c in range(C):
                oneh[:, c, c] = 1.0
            m["oneh"] = oneh
        return m

    in_maps = [make_map(b) for b in range(B)]
    # The axon terminal occasionally reports the accelerator unrecoverable
    # on the first touch after a previous process ran a NEFF. The failed
    # attempt triggers recovery terminal-side, but the local PJRT client
    # stays poisoned — tear it down between retries.
    last_err = None
    for _attempt in range(4):
        try:
            res = bass_utils.run_bass_kernel_spmd(
                nc, in_maps, core_ids=list(range(8))
            )
            break
        except Exception as e:  # noqa: BLE001
            last_err = e
            import time as _time

            _time.sleep(3.0)
            try:
                import jax

                jax.clear_caches()
                jax.extend.backend.clear_backends()
            except Exception:  # noqa: BLE001
                pass
            _time.sleep(2.0)
    else:
        raise last_err

    S1 = np.empty((B, C), np.float64)
    S2 = np.empty((B, C), np.float64)
    S3 = np.empty((B, C), np.float64)
    for b in range(B):
        r = res.results[b]
        if "out_all" in r:
            out = r["out_all"].astype(np.float64)
            S1[b] = out[:16, C]
            S3[b] = out[:16, C + 1]
            S2[b] = out[:, :C].sum(axis=0)
        else:
            s13 = r["out_s13"].astype(np.float64)
            S1[b] = s13[:, 0]
            S3[b] = s13[:, 1]
            S2[b] = r["out_acc2"].astype(np.float64).sum(axis=0)

    m1, m2, d1 = S3, S2 - S3, S1
    d2n = float(HWE) - d1
    loss = ALPHA * m1 / (d1 + SMOOTH) + (1.0 - ALPHA) * m2 / (d2n + SMOOTH)

    # active-mask: S1 != 0 implies max(target[b,c]) != 0 for non-negative
    # targets; the S1 == 0 corner is resolved exactly on host.
    active = S1 != 0.0
    for b, c in zip(*np.nonzero(~active)):
        mt = np.max(target[b, c])
        mmp = np.max(max_positiones[b, c])
        active[b, c] = not (mt == 0.0 and mmp == 0.0)

    losses = np.where(active, loss, 0.0)
    count = (losses != 0.0).sum(axis=1).astype(np.float64)
    img_losses = losses.sum(axis=1) / count
    return np.float32(img_losses.mean())

